# revision 42
# baseline (speedup 1.0000x reference)
"""Trainium2 Bass kernel for FPModule (knn_interpolate + MLP + GroupNorm).

Data-parallel over the 8 point clouds: core c owns cloud c (2048 coarse /
8192 fine points). Per core:
  Phase A: neg-d2 = 2*pf.pc - |pc|^2 via PE matmul (f32r) into PSUM
           [128 fine, 2048 coarse]; top-3 neighbors via DVE max8 + max_index
           (the per-row |pf|^2 shift doesn't change the argmax).
  Phase B: row gather of coarse features via SWDGE dma_gather; inverse-
           distance weighted sum with per-partition scalars; PE transpose to
           channel-major.
  Phase C: MLP (384->512->256, relu) + GroupNorm(32 groups of 8 channels),
           channel-major so PE contracts the channel axis; group stats and
           group-broadcast both done with tiny PE matmuls.
Output written channel-major [256, 8192] per core; host transposes back.
"""

import sys

for _p in ("/opt/trn_rl_repo",):
    if _p not in sys.path:
        sys.path.insert(0, _p)

import numpy as np

import concourse.bass as bass
import concourse.mybir as mybir
import concourse.tile as tile
from concourse import bacc
from concourse import bass_utils

f32 = mybir.dt.float32
f32r = mybir.dt.float32r
u16 = mybir.dt.uint16
i16 = mybir.dt.int16

B = 8
NC_PTS = 2048      # coarse points per cloud
NF_PTS = 8192      # fine points per cloud
NT = NF_PTS // 128  # 64 fine tiles
CH = 256           # coarse feature channels
SKIP_CH = 128
KNN = 3
GN_EPS = 1e-5
NCOL = NF_PTS // 512  # 16 col-tiles for the MLP

AluOp = mybir.AluOpType
ActFn = mybir.ActivationFunctionType


def build_program(stage=99):
    nc = bacc.Bacc("TRN2", target_bir_lowering=False, debug=False, num_devices=B)

    # ---- per-core DRAM inputs ----
    xc = nc.dram_tensor("xc", [NC_PTS, CH], f32, kind="ExternalInput")
    pcT = nc.dram_tensor("pcT", [3, NC_PTS], f32, kind="ExternalInput")
    pfT = nc.dram_tensor("pfT", [3, NF_PTS], f32, kind="ExternalInput")
    pfN = nc.dram_tensor("pfN", [NF_PTS, 3], f32, kind="ExternalInput")
    xsT = nc.dram_tensor("xsT", [SKIP_CH, NF_PTS], f32, kind="ExternalInput")
    w1t = nc.dram_tensor("w1t", [384, 512], f32, kind="ExternalInput")
    w2t = nc.dram_tensor("w2t", [512, 256], f32, kind="ExternalInput")
    b1 = nc.dram_tensor("b1", [512], f32, kind="ExternalInput")
    b2 = nc.dram_tensor("b2", [256], f32, kind="ExternalInput")
    gam = nc.dram_tensor("gam", [256], f32, kind="ExternalInput")
    bet = nc.dram_tensor("bet", [256], f32, kind="ExternalInput")
    ident = nc.dram_tensor("ident", [128, 128], f32, kind="ExternalInput")
    pc_pad = nc.dram_tensor("pc_pad", [NC_PTS, 64], f32, kind="ExternalInput")
    e8sum = nc.dram_tensor("e8sum", [128, 16], f32, kind="ExternalInput")
    e8exp = nc.dram_tensor("e8exp", [16, 128], f32, kind="ExternalInput")
    outT = nc.dram_tensor("outT", [CH, NF_PTS], f32, kind="ExternalOutput")

    with tile.TileContext(nc) as tc:
        with tc.tile_pool(name="persist", bufs=1) as pp:
            # long-lived SBUF
            ident_sb = pp.tile([128, 128], f32, tag="ident")
            nc.sync.dma_start(ident_sb[:], ident.ap())
            wn_all = pp.tile([128, NT, KNN], f32, tag="wn")
            wrap = pp.tile([128, NT, KNN, 8], u16, tag="wrap")
            yT = pp.tile([128, 2, NF_PTS], f32, tag="yT")  # interp, channel-major

            # ---------------- Phase A: distances + top-3 ----------------
            with tc.tile_pool(name="pa_sb", bufs=1) as ap_, \
                 tc.tile_pool(name="pa_ps", bufs=2, space="PSUM") as aps:
                # pcT4 = [-|pc|^2 ; 2*pc^T], pfT4 = [1 ; pf^T] so that
                # matmul(pfT4_tile, pcT4) = 2*pf.pc - |pc|^2 = negd2 + |pf|^2.
                # Engine-op APs must start at partition 0/32/64/96, so the
                # computed row sits at partition 0 and the DMA'd rows at 1-3.
                vals_all = ap_.tile([128, NT, 8], f32, tag="vals")
                idx_all = ap_.tile([128, NT, 8], u16, tag="idx")
                pcT4 = ap_.tile([4, NC_PTS], f32, tag="pcT4")
                pfT4 = ap_.tile([4, NF_PTS], f32, tag="pfT4")
                tmp3 = ap_.tile([3, NC_PTS], f32, tag="tmp3")

                nc.sync.dma_start(tmp3[:], pcT.ap())  # host supplies 2*pc^T
                nc.sync.dma_start(pcT4[1:4, :], pcT.ap())
                # row 0 = -|pc|^2 = -0.25 * sum((2pc)^2) via a tiny PE matmul
                sq3 = ap_.tile([3, NC_PTS], f32, tag="sq3")
                nc.scalar.square(sq3[:], tmp3[:])
                negq = ap_.tile([3, 1], f32, tag="negq")
                nc.vector.memset(negq[:], -0.25)
                psq = aps.tile([1, NC_PTS], f32, tag="d2")  # borrow a d2 slot
                for j in range(NC_PTS // 512):
                    nc.tensor.matmul(
                        psq[:, j * 512:(j + 1) * 512], negq[:],
                        sq3[:, j * 512:(j + 1) * 512],
                        start=True, stop=True)
                nc.scalar.copy(pcT4[0:1, :], psq[:])

                nc.vector.memset(pfT4[:], 1.0)  # row 0 stays 1.0
                nc.sync.dma_start(pfT4[1:4, :], pfT.ap())

                # fine-point coords in [128, tile, 3] layout (for the exact
                # re-rank below)
                pf_nat = ap_.tile([128, NT, 3], f32, tag="pfnat")
                nc.sync.dma_start(
                    pf_nat[:], pfN.ap().rearrange("(t p) d -> p t d", p=128))

                # ---- phase A main loop, software-pipelined with the exact
                # re-rank of the 8 candidates ----
                # The PE's fp32 matmul carries ~2^-17 relative noise, enough
                # to swap near-tie neighbors vs the reference, so gather the
                # 8 candidates' coords (dma_gather of 256B-padded rows) and
                # re-rank by exact fp32 d2. The gathers for tile group g run
                # while group g+1's d2 scans keep the DVE busy; the re-rank
                # DVE ops for group g are emitted after group g+1's scans so
                # the DVE never stalls on gather latency.
                # wrap8[16g+q, t, k, b] = idx_all[16b+q, t, k], k = 0..7
                wrap8 = ap_.tile([128, NT, 8, 8], u16, tag="wrap8")
                nd2x = ap_.tile([128, NT, 8], f32, tag="nd2x")
                svals = ap_.tile([128, NT, 8], f32, tag="svals")
                spos = ap_.tile([128, NT, 8], u16, tag="spos")
                GT = 8  # tiles per group
                g4_tiles = {}

                def rerank_group(grp):
                    for t in range(GT * grp, GT * (grp + 1)):
                        g4v = g4_tiles.pop(t)[:].rearrange(
                            "p (t k) d -> p t k d", t=1)
                        dif = ap_.tile([128, 1, 8, 3], f32, tag="dif")
                        nc.vector.tensor_sub(
                            dif[:], g4v[:, :, :, 0:3],
                            pf_nat[:, t:t + 1, :].unsqueeze(2)
                            .to_broadcast([128, 1, 8, 3]))
                        nc.scalar.square(dif[:], dif[:])
                        nc.vector.tensor_reduce(
                            nd2x[:, t:t + 1, :], dif[:],
                            axis=mybir.AxisListType.X, op=AluOp.add, negate=True)
                        nc.vector.max(svals[:, t, :], nd2x[:, t, :])
                        nc.vector.max_index(
                            spos[:, t, :], svals[:, t, :], nd2x[:, t, :])

                for grp in range(NT // GT):
                    gsl = slice(GT * grp, GT * (grp + 1))
                    for t in range(GT * grp, GT * (grp + 1)):
                        ps = aps.tile([128, NC_PTS], f32, tag="d2")
                        lhsT = pfT4[:, t * 128:(t + 1) * 128]
                        for j in range(NC_PTS // 512):
                            nc.tensor.matmul(
                                ps[:, j * 512:(j + 1) * 512], lhsT,
                                pcT4[:, j * 512:(j + 1) * 512],
                                start=True, stop=True)
                        nc.vector.max(vals_all[:, t, :], ps[:])
                        nc.vector.max_index(idx_all[:, t, :], vals_all[:, t, :], ps[:])

                    # wrapped idx layout + candidate-coord gathers for this group
                    for bq in range(8):
                        nc.sync.dma_start(
                            wrap8[0:16, gsl, :, bq],
                            idx_all[16 * bq:16 * (bq + 1), gsl, :])
                    for g in range(1, 8):
                        nc.sync.dma_start(
                            wrap8[16 * g:16 * (g + 1), gsl, :, :],
                            wrap8[0:16, gsl, :, :])
                    for t in range(GT * grp, GT * (grp + 1)):
                        g4 = ap_.tile([128, 8, 64], f32, tag="g4", bufs=2 * GT)
                        g4_tiles[t] = g4
                        nc.gpsimd.dma_gather(
                            out_ap=g4[:],
                            in_ap=pc_pad.ap(),
                            idxs_ap=wrap8[:, t, :, :].bitcast(i16),
                            num_idxs=8 * 128,
                            num_idxs_reg=8 * 128,
                            elem_size=64,
                            queue_num=0,
                        )
                    if grp > 0:
                        rerank_group(grp - 1)
                rerank_group(NT // GT - 1)


                # final_idx[p,t,k] = idx_all[p,t,spos[p,t,k]] via arithmetic
                # select (sum over j of (spos==j)*idx_all[...,j])
                sposf = ap_.tile([128, NT, KNN], f32, tag="sposf")
                nc.vector.tensor_copy(sposf[:], spos[:, :, 0:KNN])
                idxf = ap_.tile([128, NT, 8], f32, tag="idxf")
                nc.vector.tensor_copy(idxf[:], idx_all[:])
                sel = ap_.tile([128, NT, KNN], f32, tag="sel")
                nc.vector.memset(sel[:], 0.0)
                tmpsel = ap_.tile([128, NT, KNN], f32, tag="tmpsel")
                for j in range(8):
                    nc.vector.scalar_tensor_tensor(
                        tmpsel[:], sposf[:], float(j),
                        idxf[:, :, j:j + 1].to_broadcast([128, NT, KNN]),
                        op0=AluOp.is_equal, op1=AluOp.mult)
                    nc.vector.tensor_add(sel[:], sel[:], tmpsel[:])
                final_idx = ap_.tile([128, NT, KNN], u16, tag="fidx")
                nc.vector.tensor_copy(final_idx[:], sel[:])

                # ---- weights from exact d2: w = 1/clip(d2), normalized ----
                d2a = ap_.tile([128, NT, KNN], f32, tag="d2a")
                nc.vector.tensor_scalar_mul(d2a[:], svals[:, :, 0:KNN], -1.0)
                nc.vector.tensor_scalar_max(d2a[:], d2a[:], 1e-16)
                wrec = ap_.tile([128, NT, KNN], f32, tag="wrec")
                nc.vector.reciprocal(wrec[:], d2a[:])
                ssum = ap_.tile([128, NT], f32, tag="ssum")
                nc.vector.tensor_reduce(
                    ssum[:], wrec[:], axis=mybir.AxisListType.X, op=AluOp.add)
                rs = ap_.tile([128, NT], f32, tag="rs")
                nc.vector.reciprocal(rs[:], ssum[:])
                nc.vector.tensor_mul(
                    wn_all[:], wrec[:],
                    rs[:].unsqueeze(2).to_broadcast([128, NT, KNN]))

                # ---- wrapped gather-index layout for the feature gather ----
                # wrap[16g+q, t, k, b] = final_idx[16b+q, t, k]
                for bq in range(8):
                    for k in range(KNN):
                        nc.sync.dma_start(
                            wrap[0:16, :, k, bq],
                            final_idx[16 * bq:16 * (bq + 1), :, k])
                for g in range(1, 8):
                    nc.sync.dma_start(wrap[16 * g:16 * (g + 1), :, :, :], wrap[0:16, :, :, :])

            # ---------------- Phases B+C interleaved per col-tile ----------------
            with tc.tile_pool(name="bc_sb", bufs=1) as cp, \
                 tc.tile_pool(name="g_sb", bufs=8) as gp, \
                 tc.tile_pool(name="y_sb", bufs=4) as yp, \
                 tc.tile_pool(name="c_work", bufs=2) as wp, \
                 tc.tile_pool(name="pt_ps", bufs=2, space="PSUM") as ptp, \
                 tc.tile_pool(name="mm_ps", bufs=2, space="PSUM") as mp, \
                 tc.tile_pool(name="st_ps", bufs=2, space="PSUM") as sp, \
                 tc.tile_pool(name="rep_ps", bufs=2, space="PSUM") as rp:
                w1t_sb = cp.tile([128, 3, 512], f32, tag="w1t")
                nc.sync.dma_start(
                    w1t_sb[:], w1t.ap().rearrange("(k p) n -> p k n", p=128))
                w2t_sb = cp.tile([128, 4, 256], f32, tag="w2t")
                nc.sync.dma_start(
                    w2t_sb[:], w2t.ap().rearrange("(k p) n -> p k n", p=128))
                b1_sb = cp.tile([128, 4], f32, tag="b1")
                nc.sync.dma_start(b1_sb[:], b1.ap().rearrange("(c p) -> p c", p=128))
                b2_sb = cp.tile([128, 2], f32, tag="b2")
                nc.sync.dma_start(b2_sb[:], b2.ap().rearrange("(c p) -> p c", p=128))
                gam_sb = cp.tile([128, 2], f32, tag="gam")
                nc.sync.dma_start(gam_sb[:], gam.ap().rearrange("(c p) -> p c", p=128))
                bet_sb = cp.tile([128, 2], f32, tag="bet")
                nc.sync.dma_start(bet_sb[:], bet.ap().rearrange("(c p) -> p c", p=128))
                e8sum_sb = cp.tile([128, 16], f32r, tag="e8sum")
                nc.sync.dma_start(e8sum_sb[:], e8sum.ap().bitcast(f32r))
                # copies at base partition 0 and 32 (matmul needs lhsT/rhs
                # base partitions equal)
                e8exp_sb = cp.tile([48, 128], f32r, tag="e8exp")
                nc.sync.dma_start(e8exp_sb[0:16, :], e8exp.ap().bitcast(f32r))
                nc.sync.dma_start(e8exp_sb[32:48, :], e8exp.ap().bitcast(f32r))

                for n in range(NCOL):
                    # --- Phase B for the 4 fine tiles of this col-tile ---
                    for ti in range(4):
                        t = 4 * n + ti
                        g = gp.tile([128, KNN, CH], f32, tag="gath")
                        nc.gpsimd.dma_gather(
                            out_ap=g[:],
                            in_ap=xc.ap(),
                            idxs_ap=wrap[:, t, :, :].bitcast(i16),
                            num_idxs=KNN * 128,
                            num_idxs_reg=KNN * 128,
                            elem_size=CH,
                            queue_num=0,
                        )
                        y = yp.tile([128, CH], f32, tag="y")
                        nc.vector.tensor_scalar_mul(y[:], g[:, 0, :], wn_all[:, t, 0:1])
                        nc.vector.scalar_tensor_tensor(
                            y[:], g[:, 1, :], wn_all[:, t, 1:2], y[:],
                            op0=AluOp.mult, op1=AluOp.add)
                        nc.vector.scalar_tensor_tensor(
                            y[:], g[:, 2, :], wn_all[:, t, 2:3], y[:],
                            op0=AluOp.mult, op1=AluOp.add)
                        for half in range(2):
                            pt = ptp.tile([128, 128], f32, tag="pt")
                            nc.tensor.transpose(
                                pt[:], y[:, 128 * half:128 * (half + 1)], ident_sb[:])
                            nc.scalar.copy(
                                yT[:, half, t * 128:(t + 1) * 128], pt[:])

                    ncol_sl = slice(n * 512, (n + 1) * 512)
                    xs_ch = gp.tile([128, 512], f32, tag="xs")
                    nc.sync.dma_start(xs_ch[:], xsT.ap()[:, ncol_sl])
                    rhs_chunks = [yT[:, 0, ncol_sl], yT[:, 1, ncol_sl], xs_ch[:]]

                    # --- layer 1: 384 -> 512, relu ---
                    h1 = wp.tile([128, 4, 512], f32, tag="h1")
                    for m in range(4):
                        ps1 = mp.tile([128, 512], f32, tag="mm")
                        for k in range(3):
                            nc.tensor.matmul(
                                ps1[:], w1t_sb[:, k, m * 128:(m + 1) * 128],
                                rhs_chunks[k],
                                start=(k == 0), stop=(k == 2))
                        nc.scalar.activation(
                            h1[:, m, :], ps1[:], ActFn.Relu, bias=b1_sb[:, m:m + 1])

                    # --- layer 2: 512 -> 256, relu ---
                    h2 = wp.tile([128, 2, 512], f32r, tag="h2")
                    for m in range(2):
                        ps2 = mp.tile([128, 512], f32, tag="mm")
                        for k in range(4):
                            nc.tensor.matmul(
                                ps2[:], w2t_sb[:, k, m * 128:(m + 1) * 128],
                                h1[:, k, :],
                                start=(k == 0), stop=(k == 3))
                        nc.scalar.activation(
                            h2[:, m, :], ps2[:], ActFn.Relu, bias=b2_sb[:, m:m + 1])

                    # --- GroupNorm(32 groups of 8 channels) ---
                    # stats live at base partitions 0 / 32 (matmul rhs needs
                    # base_partition in {0, 32, 64})
                    sqb = wp.tile([128, 2, 512], f32r, tag="sqb")
                    stats = wp.tile([48, 512], f32, tag="stats")   # sums
                    stats2 = wp.tile([48, 512], f32, tag="stats2")  # sum of squares
                    rs8 = wp.tile([48, 512], f32r, tag="rs8")
                    negmu_rs = wp.tile([48, 512], f32r, tag="negmurs")
                    for m in range(2):
                        sl = slice(32 * m, 32 * m + 16)
                        nc.vector.tensor_mul(sqb[:, m, :], h2[:, m, :].bitcast(f32), h2[:, m, :].bitcast(f32))
                        psS = sp.tile([16, 512], f32, tag="st")
                        nc.tensor.matmul(
                            psS[:], e8sum_sb[:],
                            h2[:, m, :], start=True, stop=True)
                        nc.vector.tensor_copy(stats[sl, :], psS[:])
                        psQ = sp.tile([16, 512], f32, tag="st")
                        nc.tensor.matmul(
                            psQ[:], e8sum_sb[:],
                            sqb[:, m, :], start=True, stop=True)
                        nc.vector.tensor_copy(stats2[sl, :], psQ[:])

                        # 64*var = 8*sum2 - sum^2 ; rs8 = sqrt(64/(64var+64eps))
                        # scratch tiles span [48, 512] so slices share the
                        # base partition of stats/rs8 (walrus requires equal
                        # SB base partitions for 2-input DVE ops)
                        sqs = wp.tile([48, 512], f32, tag="sqs")
                        nc.scalar.square(sqs[sl, :], stats[sl, :])
                        v64 = wp.tile([48, 512], f32, tag="v64")
                        nc.vector.scalar_tensor_tensor(
                            v64[sl, :], stats2[sl, :], 8.0, sqs[sl, :],
                            op0=AluOp.mult, op1=AluOp.subtract)
                        nc.vector.tensor_scalar_add(v64[sl, :], v64[sl, :], 64.0 * GN_EPS)
                        rcp = wp.tile([48, 512], f32, tag="rcp")
                        nc.vector.reciprocal(rcp[sl, :], v64[sl, :])
                        nc.scalar.activation(rs8[sl, :], rcp[sl, :], ActFn.Sqrt, scale=64.0)
                        nc.vector.scalar_tensor_tensor(
                            negmu_rs[sl, :], stats[sl, :], -0.125,
                            rs8[sl, :].bitcast(f32),
                            op0=AluOp.mult, op1=AluOp.mult)

                    outsb = wp.tile([128, 2, 512], f32, tag="outsb")
                    for m in range(2):
                        sl = slice(32 * m, 32 * m + 16)
                        pA = rp.tile([128, 512], f32, tag="rep")
                        nc.tensor.matmul(
                            pA[:], e8exp_sb[sl, :],
                            rs8[sl, :],
                            start=True, stop=True)
                        pB = rp.tile([128, 512], f32, tag="rep")
                        nc.tensor.matmul(
                            pB[:], e8exp_sb[sl, :],
                            negmu_rs[sl, :],
                            start=True, stop=True)
                        tmpa = wp.tile([128, 512], f32, tag="tmpa")
                        nc.vector.tensor_mul(tmpa[:], h2[:, m, :].bitcast(f32), pA[:])
                        nc.vector.tensor_add(tmpa[:], tmpa[:], pB[:])
                        nc.vector.scalar_tensor_tensor(
                            outsb[:, m, :], tmpa[:], gam_sb[:, m:m + 1],
                            bet_sb[:, m:m + 1].to_broadcast([128, 512]),
                            op0=AluOp.mult, op1=AluOp.add)
                        nc.sync.dma_start(
                            outT.ap()[128 * m:128 * (m + 1), ncol_sl], outsb[:, m, :])

    nc.compile()
    return nc


_NC_CACHE = None


def _get_program():
    global _NC_CACHE
    if _NC_CACHE is None:
        _NC_CACHE = build_program()
    return _NC_CACHE


def make_in_maps(x, pos, x_skip, pos_skip, W1, b1, W2, b2, gamma2, beta2):
    x = np.ascontiguousarray(np.asarray(x, np.float32))
    pos = np.asarray(pos, np.float32)
    x_skip = np.asarray(x_skip, np.float32)
    pos_skip = np.asarray(pos_skip, np.float32)

    xc = x.reshape(B, NC_PTS, CH)
    pcT = np.ascontiguousarray(2.0 * pos.reshape(B, NC_PTS, 3).transpose(0, 2, 1))
    pfT = np.ascontiguousarray(pos_skip.reshape(B, NF_PTS, 3).transpose(0, 2, 1))
    pfN = np.ascontiguousarray(pos_skip.reshape(B, NF_PTS, 3))
    xsT = np.ascontiguousarray(x_skip.reshape(B, NF_PTS, SKIP_CH).transpose(0, 2, 1))

    w1t = np.ascontiguousarray(np.asarray(W1, np.float32).T)  # [384, 512]
    w2t = np.ascontiguousarray(np.asarray(W2, np.float32).T)  # [512, 256]
    ident = np.eye(128, dtype=np.float32)
    pc_pad_all = np.zeros((B, NC_PTS, 64), np.float32)
    pc_pad_all[:, :, :3] = pos.reshape(B, NC_PTS, 3)
    grp = np.arange(128) // 8
    e8sum = (grp[:, None] == np.arange(16)[None, :]).astype(np.float32)  # [128,16]
    e8exp = np.ascontiguousarray(e8sum.T)  # [16,128]

    common = dict(
        w1t=w1t, w2t=w2t,
        b1=np.asarray(b1, np.float32), b2=np.asarray(b2, np.float32),
        gam=np.asarray(gamma2, np.float32), bet=np.asarray(beta2, np.float32),
        ident=ident, e8sum=e8sum, e8exp=e8exp,
    )
    in_maps = []
    for c in range(B):
        m = dict(common)
        m["xc"] = np.ascontiguousarray(xc[c])
        m["pc_pad"] = pc_pad_all[c]
        m["pcT"] = pcT[c]
        m["pfT"] = pfT[c]
        m["pfN"] = pfN[c]
        m["xsT"] = xsT[c]
        in_maps.append(m)
    return in_maps


def kernel(x, pos, reflectance, batch, x_skip, pos_skip, reflectance_skip,
           batch_skip, W1, b1, W2, b2, gamma2, beta2):
    nc = _get_program()
    in_maps = make_in_maps(x, pos, x_skip, pos_skip, W1, b1, W2, b2, gamma2, beta2)
    res = bass_utils.run_bass_kernel_spmd(nc, in_maps, core_ids=list(range(B)))
    h = np.concatenate(
        [np.ascontiguousarray(res.results[c]["outT"].T) for c in range(B)], axis=0)
    return (
        h,
        np.asarray(pos_skip),
        np.asarray(reflectance_skip),
        np.asarray(batch_skip),
    )
